# revision 21
# baseline (speedup 1.0000x reference)
"""Trainium2 Bass kernel for the ActorCritic ragged-sequence problem.

Strategy
--------
Data-parallel over batch B=64 across 8 NeuronCores (8 batch rows per core,
weights replicated, no collectives; per-core (8,5) outputs are concatenated on
the host).

Per core the dominant work is the position-actor pair-MLP:
    h[b,t] = relu(x_t @ W1a + x_{t+1} @ W1b + b1p);  scores[b,t] = w2p . h[b,t]
computed as weight-stationary matmuls over the flattened 8192 rows:
  - states are passed host-transposed (feature-major); a casting SWDGE DMA
    loads them straight into persistent bf16 X^T strips, so the moving
    operand slices are just free-dim windows and the +1 shift of the
    "second" element of each pair is a one-element slice offset — the PE
    accumulates u_t + v_{t+1} in PSUM for free.
  - the row space is processed in quarters of 4x512 rows so each 128x128
    stationary weight tile is loaded once per 4 matmuls (LDWEIGHTS reuse).
  - ACT applies bias+relu per chan-tile; the w2p dot runs as M=1 matmuls at
    quarter end (LDWEIGHTS is 1 column there, ~free).
Masked log-softmax + entropy run on an (8, 1024) batch-major score tile; the
additive length mask is folded into the PSUM->SBUF strip copy.
Index-derived tensors (masks, one-hots, gathered pair embeddings e1/e2) are
computed on the host from the actual inputs at call time — pure
indexing/layout, no FLOPs moved off-device.  The symbol head and critic are
emitted first so their matmuls fill the PE while the big DMAs stream in.
"""

import os
import numpy as np

B, S, E, A = 64, 1024, 512, 128
NCORES = 8
BC = B // NCORES          # batch rows per core
H = 2 * E                 # pair-MLP hidden dim
R = BC * S                # flattened rows per core
RS = 512                  # row-slice (matmul moving free dim)
NRS = R // RS             # 16 row slices
NQ = 8                    # row-slice groups ("quarters")
QS = NRS // NQ            # row slices per group
KT = E // 128             # 4 k-tiles over the E features
CT = H // 128             # 8 chan tiles of the hidden dim
XTP = R + 8               # padded free dim of the transposed states

MODE = os.environ.get("K_MODE", "bf16")
TRACE = os.environ.get("K_TRACE", "1") == "1"

LAST_EXEC_NS = None
_CACHED = {}

_LDWOPT = os.environ.get("K_LDWOPT", "0") == "1"
_PATCHED = False


def _patch_walrus_flags():
    """Re-enable walrus LDWEIGHTS dedup (repeated stationary operands) for
    this process's compiles."""
    global _PATCHED
    if _PATCHED or not _LDWOPT:
        return
    import concourse.bass_utils as _bu

    _orig = _bu.run_command

    def _rc(argv, **kw):
        argv = [
            "--enable-ldw-opt=true" if a == "--enable-ldw-opt=false" else a
            for a in argv
        ]
        return _orig(argv, **kw)

    _bu.run_command = _rc
    _PATCHED = True


def _build(mode):
    import concourse.tile as tile
    from concourse import bacc, mybir

    _patch_walrus_flags()

    F32 = mybir.dt.float32
    BF16 = mybir.dt.bfloat16
    CD = BF16
    AF = mybir.ActivationFunctionType
    OP = mybir.AluOpType
    AX = mybir.AxisListType

    nc = bacc.Bacc("TRN2", target_bir_lowering=False, debug=False)

    # ---- DRAM parameters -------------------------------------------------
    F8 = mybir.dt.float8e4
    K2 = KT // 2              # 256-deep fp8 DoubleRow k-tiles
    xt_d = nc.dram_tensor("xt", [KT, 128, XTP], F32, kind="ExternalInput")
    if mode == "fp8":
        wa_d = nc.dram_tensor("wa8", [K2, 128, 2, H], F8, kind="ExternalInput")
        wb_d = nc.dram_tensor("wb8", [K2, 128, 2, H], F8, kind="ExternalInput")
    else:
        wa_d = nc.dram_tensor("wa", [KT, 128, H], CD, kind="ExternalInput")
        wb_d = nc.dram_tensor("wb", [KT, 128, H], CD, kind="ExternalInput")
    if mode == "fp8":
        w2p_d = nc.dram_tensor("w2p8", [128, 2, 16], F8, kind="ExternalInput")
    else:
        w2p_d = nc.dram_tensor("w2p_t", [128, CT], CD, kind="ExternalInput")
    b1p_d = nc.dram_tensor("b1p_t", [128, CT], F32, kind="ExternalInput")
    mask_d = nc.dram_tensor("addmask", [BC, S], F32, kind="ExternalInput")
    paoh_d = nc.dram_tensor("pa_onehot", [BC, S], F32, kind="ExternalInput")
    e12_d = nc.dram_tensor("e12t", [CT, 128, BC], CD, kind="ExternalInput")
    ws_d = nc.dram_tensor("ws", [CT, 128, H], CD, kind="ExternalInput")
    b1s_d = nc.dram_tensor("b1s_t", [128, CT], F32, kind="ExternalInput")
    w2s_d = nc.dram_tensor("w2s", [CT, 128, A], CD, kind="ExternalInput")
    b2s_d = nc.dram_tensor("b2s_row", [1, A], CD, kind="ExternalInput")
    soh_d = nc.dram_tensor("sym_onehot", [BC, A], F32, kind="ExternalInput")
    clst_d = nc.dram_tensor("clst", [KT, 128, BC], CD, kind="ExternalInput")
    wc1_d = nc.dram_tensor("wc1", [KT, 128, E], CD, kind="ExternalInput")
    bc1_d = nc.dram_tensor("bc1_t", [128, KT], F32, kind="ExternalInput")
    wc2_d = nc.dram_tensor("wc2_t", [128, KT], CD, kind="ExternalInput")
    bc2_d = nc.dram_tensor("bc2_col", [BC, 1], F32, kind="ExternalInput")
    out_d = nc.dram_tensor("out", [BC, 5], F32, kind="ExternalOutput")

    VCT = E // 128  # chan tiles of the critic hidden dim (4)

    with tile.TileContext(nc) as tc:
        with (
            tc.tile_pool(name="weights", bufs=1) as wpool,
            tc.tile_pool(name="hbuf", bufs=1) as hpool,
            tc.tile_pool(name="small", bufs=1) as spool,
            tc.tile_pool(name="strips", bufs=2) as stpool,
            tc.tile_pool(name="psmain", bufs=2, space="PSUM") as psmain,
            tc.tile_pool(name="pssc", bufs=2, space="PSUM") as pssc,
            tc.tile_pool(name="ps3", bufs=2, space="PSUM") as ps3,
        ):
            # ---- symbol head + critic inputs first (their matmuls fill the
            # PE while the big state/weight DMAs stream in) ----------------
            ws_sb = [wpool.tile([128, H], CD, name=f"ws{k}") for k in range(CT)]
            w2s_sb = [wpool.tile([128, A], CD, name=f"w2s{k}") for k in range(CT)]
            e12_sb = [wpool.tile([128, BC], CD, name=f"e12{k}") for k in range(CT)]
            for k in range(CT):
                nc.sync.dma_start(e12_sb[k][:], e12_d[k, :, :])
                nc.sync.dma_start(ws_sb[k][:], ws_d[k, :, :])
                nc.sync.dma_start(w2s_sb[k][:], w2s_d[k, :, :])
            b1s_sb = wpool.tile([128, CT], F32, name="b1s")
            b2s_sb = wpool.tile([1, A], CD, name="b2s")
            soh_sb = wpool.tile([BC, A], F32, name="soh")
            nc.sync.dma_start(b1s_sb[:], b1s_d[:, :])
            nc.sync.dma_start(b2s_sb[:], b2s_d[:, :])
            nc.sync.dma_start(soh_sb[:], soh_d[:, :])
            clst_sb = [wpool.tile([128, BC], CD, name=f"cls{k}") for k in range(KT)]
            wc1_sb = [wpool.tile([128, E], CD, name=f"wc1{k}") for k in range(KT)]
            for k in range(KT):
                nc.sync.dma_start(clst_sb[k][:], clst_d[k, :, :])
                nc.sync.dma_start(wc1_sb[k][:], wc1_d[k, :, :])
            bc1_sb = wpool.tile([128, KT], F32, name="bc1")
            wc2_sb = wpool.tile([128, KT], CD, name="wc2")
            bc2_sb = wpool.tile([BC, 1], F32, name="bc2")
            nc.sync.dma_start(bc1_sb[:], bc1_d[:, :])
            nc.sync.dma_start(wc2_sb[:], wc2_d[:, :])
            nc.sync.dma_start(bc2_sb[:], bc2_d[:, :])
            ones_sb = wpool.tile([1, BC], CD, name="ones")
            nc.vector.memset(ones_sb[:], 1.0)

            outbuf = spool.tile([BC, 5], F32, name="outbuf")
            nc.vector.memset(outbuf[:], 0.0)

            # ---- symbol head ---------------------------------------------
            sh_sb = [spool.tile([128, BC], CD, name=f"sh{ct}") for ct in range(CT)]
            for ct in range(CT):
                p3 = ps3.tile([128, BC], F32, name="p3", tag="p3")
                for k in range(CT):
                    nc.tensor.matmul(
                        p3[:],
                        ws_sb[k][:, ct * 128 : (ct + 1) * 128],
                        e12_sb[k][:],
                        start=(k == 0),
                        stop=(k == CT - 1),
                    )
                nc.scalar.activation(
                    sh_sb[ct][:], p3[:], AF.Relu, bias=b1s_sb[:, ct : ct + 1]
                )
            psl = ps3.tile([BC, A], F32, name="psl", tag="p3")
            for ct in range(CT):
                nc.tensor.matmul(
                    psl[:], sh_sb[ct][:], w2s_sb[ct][:], start=(ct == 0), stop=False
                )
            nc.tensor.matmul(
                psl[:], ones_sb[:], b2s_sb[:], start=False, stop=True
            )
            smy = spool.tile([BC, A], F32, name="smy")
            nc.vector.tensor_copy(smy[:], psl[:])
            mny = spool.tile([BC, 1], F32, name="mny")
            nc.vector.tensor_reduce(mny[:], smy[:], axis=AX.X, op=OP.max, negate=True)
            pey = spool.tile([BC, A], F32, name="pey")
            zsy = spool.tile([BC, 1], F32, name="zsy")
            nc.scalar.activation(
                pey[:], smy[:], AF.Exp, bias=mny[:, 0:1], accum_out=zsy[:]
            )
            p2y = spool.tile([BC, A], F32, name="p2y")
            s2y = spool.tile([BC, 1], F32, name="s2y")
            nc.vector.tensor_mul(p2y[:], pey[:], smy[:])
            nc.vector.tensor_reduce(s2y[:], p2y[:], axis=AX.X, op=OP.add)
            lzy = spool.tile([BC, 1], F32, name="lzy")
            nc.scalar.activation(lzy[:], zsy[:], AF.Ln)
            lsey = spool.tile([BC, 1], F32, name="lsey")
            nc.vector.tensor_sub(lsey[:], lzy[:], mny[:])
            tmpy = spool.tile([BC, A], F32, name="tmpy")
            say = spool.tile([BC, 1], F32, name="say")
            nc.vector.tensor_mul(tmpy[:], smy[:], soh_sb[:])
            nc.vector.tensor_reduce(say[:], tmpy[:], axis=AX.X, op=OP.add)
            rzy = spool.tile([BC, 1], F32, name="rzy")
            nc.vector.reciprocal(rzy[:], zsy[:])
            s2zy = spool.tile([BC, 1], F32, name="s2zy")
            nc.vector.tensor_mul(s2zy[:], s2y[:], rzy[:])
            nc.vector.tensor_sub(outbuf[:, 1:2], say[:], lsey[:])   # logp_sym
            nc.vector.tensor_sub(outbuf[:, 4:5], lsey[:], s2zy[:])  # ent_sym

            # ---- critic ---------------------------------------------------
            hc_sb = [spool.tile([128, BC], CD, name=f"hc{ct}") for ct in range(VCT)]
            for ct in range(VCT):
                pc = ps3.tile([128, BC], F32, name="pc", tag="p3")
                for k in range(KT):
                    nc.tensor.matmul(
                        pc[:],
                        wc1_sb[k][:, ct * 128 : (ct + 1) * 128],
                        clst_sb[k][:],
                        start=(k == 0),
                        stop=(k == KT - 1),
                    )
                nc.scalar.activation(
                    hc_sb[ct][:], pc[:], AF.Relu, bias=bc1_sb[:, ct : ct + 1]
                )
            pv = ps3.tile([BC, 1], F32, name="pv", tag="p3")
            for ct in range(VCT):
                nc.tensor.matmul(
                    pv[:], hc_sb[ct][:], wc2_sb[:, ct : ct + 1],
                    start=(ct == 0), stop=(ct == VCT - 1),
                )
            nc.vector.tensor_add(outbuf[:, 2:3], pv[:], bc2_sb[:])  # val

            # ---- main-path inputs ----------------------------------------
            if mode == "fp8":
                wa_sb = [wpool.tile([128, 2, H], F8, name=f"wa{k}") for k in range(K2)]
                wb_sb = [wpool.tile([128, 2, H], F8, name=f"wb{k}") for k in range(K2)]
                for k in range(K2):
                    nc.sync.dma_start(wa_sb[k][:], wa_d[k, :, :, :])
                    nc.sync.dma_start(wb_sb[k][:], wb_d[k, :, :, :])
            else:
                wa_sb = [wpool.tile([128, H], CD, name=f"wa{k}") for k in range(KT)]
                wb_sb = [wpool.tile([128, H], CD, name=f"wb{k}") for k in range(KT)]
                for k in range(KT):
                    nc.sync.dma_start(wa_sb[k][:], wa_d[k, :, :])
                    nc.sync.dma_start(wb_sb[k][:], wb_d[k, :, :])
            if mode == "fp8":
                w2p_sb = wpool.tile([128, 2, 16], F8, name="w2p")
                nc.sync.dma_start(w2p_sb[:], w2p_d[:, :, :])
            else:
                w2p_sb = wpool.tile([128, CT], CD, name="w2p")
                nc.sync.dma_start(w2p_sb[:], w2p_d[:, :])
            b1p_sb = wpool.tile([128, CT], F32, name="b1p")
            nc.sync.dma_start(b1p_sb[:], b1p_d[:, :])
            mask_sb = wpool.tile([BC, S], F32, name="mask")
            paoh_sb = wpool.tile([BC, S], F32, name="paoh")
            nc.sync.dma_start(mask_sb[:], mask_d[:, :])
            nc.sync.dma_start(paoh_sb[:], paoh_d[:, :])

            # persistent bf16 X^T strips loaded by casting SWDGE DMA, one
            # independent tile per (k, quarter) window (2049 columns: the
            # extra boundary column serves the +1-shifted V operand) so each
            # quarter's matmuls depend only on its own four window DMAs.
            CW = R // NQ  # 2048 columns per window
            xbf = {}
            if mode == "fp8":
                from concourse.tile_rust import add_dep_helper

                XW = CW + 16  # pad the plane stride to a 16-byte multiple
                prev_dma = {}
                for q in range(NQ):
                    for k2 in range(K2):
                        t = wpool.tile([128, 2, XW], F8, name=f"x8_{k2}_{q}")
                        for j in range(2):
                            dma = nc.gpsimd.dma_start(
                                t[:, j, 0 : CW + 1],
                                xt_d[2 * k2 + j, :, q * CW : q * CW + CW + 1],
                            )
                            # serialize each (k2, j) lane across quarters so
                            # quarter 0's windows finish first instead of all
                            # windows round-robining to a simultaneous finish
                            if (k2, j) in prev_dma:
                                add_dep_helper(
                                    dma.ins, prev_dma[(k2, j)].ins, True,
                                    "x window quarter ordering",
                                )
                            prev_dma[(k2, j)] = dma
                        xbf[(k2, q)] = t
            else:
                for q in range(NQ):
                    for k in range(KT):
                        t = wpool.tile([128, CW + 1], CD, name=f"xbf{k}_{q}")
                        nc.gpsimd.dma_start(
                            t[:], xt_d[k, :, q * CW : q * CW + CW + 1]
                        )
                        xbf[(k, q)] = t

            scores_sb = wpool.tile([BC, S], F32, name="scores")

            # ---- main pair-MLP: quarters of 4 row slices ------------------
            for q in range(NQ):
                ps_q = [
                    psmain.tile([128, RS], F32, name=f"ps{j}", tag=f"ps{j}")
                    for j in range(QS)
                ]
                hs = {}
                for ct in range(CT):
                    if mode == "fp8":
                        for w in range(2 * K2):
                            ab, k2 = divmod(w, K2)
                            wsb = (wa_sb if ab == 0 else wb_sb)[k2]
                            for j in range(QS):
                                nc.tensor.matmul(
                                    ps_q[j][:],
                                    wsb[:, :, ct * 128 : (ct + 1) * 128],
                                    xbf[(k2, q)][:, :, j * RS + ab : j * RS + ab + RS],
                                    start=(w == 0),
                                    stop=(w == 2 * K2 - 1),
                                    perf_mode=mybir.MatmulPerfMode.DoubleRow,
                                )
                    else:
                        for w in range(2 * KT):
                            ab, k = divmod(w, KT)
                            wsb = (wa_sb if ab == 0 else wb_sb)[k]
                            for j in range(QS):
                                nc.tensor.matmul(
                                    ps_q[j][:],
                                    wsb[:, ct * 128 : (ct + 1) * 128],
                                    xbf[(k, q)][:, j * RS + ab : j * RS + ab + RS],
                                    start=(w == 0),
                                    stop=(w == 2 * KT - 1),
                                )
                    for j in range(QS):
                        if mode == "fp8":
                            m, jj = divmod(ct, 2)
                            key = (m, j)
                            if key not in hs:
                                hs[key] = hpool.tile(
                                    [128, 2, RS], F8, name=f"h8_{m}_{j}",
                                    tag=f"h8_{m}_{j}",
                                )
                            plane = hs[key][:, jj, :]
                            nc.vector.tensor_scalar(
                                plane, ps_q[j][:],
                                b1p_sb[:, ct : ct + 1], 0.0,
                                OP.add, OP.max,
                            )
                        else:
                            h = hpool.tile([128, RS], CD, name=f"h{ct}_{j}",
                                           tag=f"h{ct}_{j}")
                            nc.scalar.activation(
                                h[:], ps_q[j][:], AF.Relu,
                                bias=b1p_sb[:, ct : ct + 1],
                            )
                            hs[(ct, j)] = h
                for j in range(QS):
                    rs = QS * q + j
                    psd = pssc.tile([1, RS], F32, name="psd", tag="psd")
                    if mode == "fp8":
                        for m in range(CT // 2):
                            nc.tensor.matmul(
                                psd[:],
                                w2p_sb[:, :, m : m + 1],
                                hs[(m, j)][:, :, :],
                                start=(m == 0),
                                stop=(m == CT // 2 - 1),
                                perf_mode=mybir.MatmulPerfMode.DoubleRow,
                            )
                    else:
                        for ct in range(CT):
                            nc.tensor.matmul(
                                psd[:],
                                w2p_sb[:, ct : ct + 1],
                                hs[(ct, j)][:],
                                start=(ct == 0),
                                stop=(ct == CT - 1),
                            )
                    sstrip = stpool.tile([1, RS], F32, name="sstrip", tag="sstrip")
                    nc.scalar.activation(
                        sstrip[:], psd[:], AF.Copy, bias=0.0,
                        scale=(1.0 / 8192.0 if mode == "fp8" else 1.0),
                    )
                    b, half = rs // 2, rs % 2
                    nc.sync.dma_start(
                        scores_sb[b : b + 1, half * RS : (half + 1) * RS], sstrip[:]
                    )

            # ---- masked log-softmax + entropy over positions -------------
            sm = spool.tile([BC, S], F32, name="sm")
            nc.vector.tensor_add(sm[:], scores_sb[:], mask_sb[:])
            mneg = spool.tile([BC, 1], F32, name="mneg")
            nc.vector.tensor_reduce(mneg[:], sm[:], axis=AX.X, op=OP.max, negate=True)
            pexp = spool.tile([BC, S], F32, name="pexp")
            zsum = spool.tile([BC, 1], F32, name="zsum")
            nc.scalar.activation(
                pexp[:], sm[:], AF.Exp, bias=mneg[:, 0:1], accum_out=zsum[:]
            )
            ps2 = spool.tile([BC, S], F32, name="ps2")
            s2 = spool.tile([BC, 1], F32, name="s2")
            nc.vector.tensor_mul(ps2[:], pexp[:], sm[:])
            nc.vector.tensor_reduce(s2[:], ps2[:], axis=AX.X, op=OP.add)
            logz = spool.tile([BC, 1], F32, name="logz")
            nc.scalar.activation(logz[:], zsum[:], AF.Ln)
            lse = spool.tile([BC, 1], F32, name="lse")
            nc.vector.tensor_sub(lse[:], logz[:], mneg[:])  # logz + max
            tmp = spool.tile([BC, S], F32, name="tmp")
            spa = spool.tile([BC, 1], F32, name="spa")
            nc.vector.tensor_mul(tmp[:], sm[:], paoh_sb[:])
            nc.vector.tensor_reduce(spa[:], tmp[:], axis=AX.X, op=OP.add)
            rz = spool.tile([BC, 1], F32, name="rz")
            nc.vector.reciprocal(rz[:], zsum[:])
            s2z = spool.tile([BC, 1], F32, name="s2z")
            nc.vector.tensor_mul(s2z[:], s2[:], rz[:])
            nc.vector.tensor_sub(outbuf[:, 0:1], spa[:], lse[:])   # logp_pos
            nc.vector.tensor_sub(outbuf[:, 3:4], lse[:], s2z[:])   # ent_pos

            nc.sync.dma_start(out_d[:, :], outbuf[:])

    nc.compile()
    return nc


def _to_cd(arr):
    import ml_dtypes

    return np.ascontiguousarray(arr).astype(ml_dtypes.bfloat16)


FP8_WSCALE = 32.0   # power-of-two prescale keeping fp8 W1p values mid-range
FP8_W2SCALE = 256.0  # prescale for w2p in fp8; scores divided by 32*256 on chip


def _to_f8(arr):
    import ml_dtypes

    return np.ascontiguousarray(arr).astype(ml_dtypes.float8_e4m3)


def kernel(**inputs):
    global LAST_EXEC_NS
    from concourse.bass_utils import run_bass_kernel_spmd

    mode = MODE
    f32 = np.float32
    states = np.asarray(inputs["states"], f32)
    cls_token = np.asarray(inputs["cls_token"], f32)
    W1p = np.asarray(inputs["W1p"], f32)
    b1p = np.asarray(inputs["b1p"], f32)
    w2p = np.asarray(inputs["w2p"], f32)
    W1s = np.asarray(inputs["W1s"], f32)
    b1s = np.asarray(inputs["b1s"], f32)
    W2s = np.asarray(inputs["W2s"], f32)
    b2s = np.asarray(inputs["b2s"], f32)
    Wc1 = np.asarray(inputs["Wc1"], f32)
    bc1 = np.asarray(inputs["bc1"], f32)
    wc2 = np.asarray(inputs["wc2"], f32)
    bc2 = np.asarray(inputs["bc2"], f32)
    lengths = np.asarray(inputs["lengths"])
    position_action = np.asarray(inputs["position_action"])
    symbol_action = np.asarray(inputs["symbol_action"])

    shared = {}
    if mode == "fp8":
        # DoubleRow layout: [k2, p, j, m] = W[256*k2 + 128*j + p, m] * S
        wa4 = W1p[:E].reshape(KT // 2, 2, 128, H).transpose(0, 2, 1, 3)
        wb4 = W1p[E:].reshape(KT // 2, 2, 128, H).transpose(0, 2, 1, 3)
        shared["wa8"] = _to_f8(wa4 * FP8_WSCALE)
        shared["wb8"] = _to_f8(wb4 * FP8_WSCALE)
        w2pm = np.zeros((128, 2, 16), np.float32)  # plane stride padded to 16B
        w2pm[:, :, : CT // 2] = w2p.reshape(CT // 2, 2, 128).transpose(2, 1, 0)
        shared["w2p8"] = _to_f8(w2pm * FP8_W2SCALE)
        shared["b1p_t"] = np.ascontiguousarray(
            b1p.reshape(CT, 128).T * FP8_WSCALE, dtype=f32
        )
    else:
        shared["wa"] = _to_cd(W1p[:E].reshape(KT, 128, H))
        shared["wb"] = _to_cd(W1p[E:].reshape(KT, 128, H))
        shared["w2p_t"] = _to_cd(w2p.reshape(CT, 128).T)
        shared["b1p_t"] = np.ascontiguousarray(b1p.reshape(CT, 128).T, dtype=f32)
    shared.update({
        "ws": _to_cd(W1s.reshape(CT, 128, H)),
        "b1s_t": np.ascontiguousarray(b1s.reshape(CT, 128).T, dtype=f32),
        "w2s": _to_cd(W2s.reshape(CT, 128, A)),
        "b2s_row": _to_cd(b2s.reshape(1, A)),
        "wc1": _to_cd(Wc1.reshape(KT, 128, E)),
        "bc1_t": np.ascontiguousarray(bc1.reshape(KT, 128).T, dtype=f32),
        "wc2_t": _to_cd(wc2.reshape(KT, 128).T),
        "bc2_col": np.full((BC, 1), bc2[0], dtype=f32),
    })

    in_maps = []
    bidx = np.arange(BC)
    tpos = np.arange(S)
    for c in range(NCORES):
        sl = slice(c * BC, (c + 1) * BC)
        st = states[sl]                       # (BC, S, E)
        xt = np.zeros((E, XTP), f32)
        xt[:, :R] = st.reshape(R, E).T
        ln = lengths[sl].astype(np.int64)
        pa = position_action[sl].astype(np.int64)
        sa = symbol_action[sl].astype(np.int64)
        addmask = np.where(tpos[None, :] < (ln - 1)[:, None], 0.0, -1e30)
        pa_onehot = np.zeros((BC, S), f32)
        pa_onehot[bidx, pa] = 1.0
        sym_onehot = np.zeros((BC, A), f32)
        sym_onehot[bidx, sa] = 1.0
        e12 = np.concatenate([st[bidx, pa], st[bidx, pa + 1]], axis=1)  # (BC, 2E)
        m = dict(shared)
        m["xt"] = np.ascontiguousarray(xt.reshape(KT, 128, XTP))
        m["addmask"] = np.ascontiguousarray(addmask, dtype=f32)
        m["pa_onehot"] = pa_onehot
        m["sym_onehot"] = sym_onehot
        m["e12t"] = _to_cd(e12.T.reshape(CT, 128, BC))
        m["clst"] = _to_cd(cls_token[sl].T.reshape(KT, 128, BC))
        in_maps.append(m)

    if mode not in _CACHED:
        _CACHED[mode] = _build(mode)
    nc = _CACHED[mode]

    res = run_bass_kernel_spmd(
        nc, in_maps, core_ids=list(range(NCORES)), trace=TRACE
    )
    LAST_EXEC_NS = res.exec_time_ns

    outs = [np.asarray(res.results[c]["out"]) for c in range(NCORES)]
    full = np.concatenate(outs, axis=0)        # (64, 5)
    return np.ascontiguousarray(full.T, dtype=f32)  # (5, 64)


# revision 22
# speedup vs baseline: 1.0304x; 1.0304x over previous
"""Trainium2 Bass kernel for the ActorCritic ragged-sequence problem.

Strategy
--------
Data-parallel over batch B=64 across 8 NeuronCores (8 batch rows per core,
weights replicated, no collectives; per-core (8,5) outputs are concatenated on
the host).

Per core the dominant work is the position-actor pair-MLP:
    h[b,t] = relu(x_t @ W1a + x_{t+1} @ W1b + b1p);  scores[b,t] = w2p . h[b,t]
computed as weight-stationary matmuls over the flattened 8192 rows:
  - states are passed host-transposed (feature-major); a casting SWDGE DMA
    loads them straight into persistent bf16 X^T strips, so the moving
    operand slices are just free-dim windows and the +1 shift of the
    "second" element of each pair is a one-element slice offset — the PE
    accumulates u_t + v_{t+1} in PSUM for free.
  - the row space is processed in quarters of 4x512 rows so each 128x128
    stationary weight tile is loaded once per 4 matmuls (LDWEIGHTS reuse).
  - ACT applies bias+relu per chan-tile; the w2p dot runs as M=1 matmuls at
    quarter end (LDWEIGHTS is 1 column there, ~free).
Masked log-softmax + entropy run on an (8, 1024) batch-major score tile; the
additive length mask is folded into the PSUM->SBUF strip copy.
Index-derived tensors (masks, one-hots, gathered pair embeddings e1/e2) are
computed on the host from the actual inputs at call time — pure
indexing/layout, no FLOPs moved off-device.  The symbol head and critic are
emitted first so their matmuls fill the PE while the big DMAs stream in.
"""

import os
import numpy as np

B, S, E, A = 64, 1024, 512, 128
NCORES = 8
BC = B // NCORES          # batch rows per core
H = 2 * E                 # pair-MLP hidden dim
R = BC * S                # flattened rows per core
RS = 512                  # row-slice (matmul moving free dim)
NRS = R // RS             # 16 row slices
NQ = 8                    # row-slice groups ("quarters")
QS = NRS // NQ            # row slices per group
KT = E // 128             # 4 k-tiles over the E features
CT = H // 128             # 8 chan tiles of the hidden dim
XTP = R + 8               # padded free dim of the transposed states

MODE = os.environ.get("K_MODE", "bf16")
TRACE = os.environ.get("K_TRACE", "1") == "1"

LAST_EXEC_NS = None
_CACHED = {}

_LDWOPT = os.environ.get("K_LDWOPT", "0") == "1"
_PATCHED = False


def _patch_walrus_flags():
    """Re-enable walrus LDWEIGHTS dedup (repeated stationary operands) for
    this process's compiles."""
    global _PATCHED
    if _PATCHED or not _LDWOPT:
        return
    import concourse.bass_utils as _bu

    _orig = _bu.run_command

    def _rc(argv, **kw):
        argv = [
            "--enable-ldw-opt=true" if a == "--enable-ldw-opt=false" else a
            for a in argv
        ]
        return _orig(argv, **kw)

    _bu.run_command = _rc
    _PATCHED = True


def _build(mode):
    import concourse.tile as tile
    from concourse import bacc, mybir

    _patch_walrus_flags()

    F32 = mybir.dt.float32
    BF16 = mybir.dt.bfloat16
    CD = BF16
    AF = mybir.ActivationFunctionType
    OP = mybir.AluOpType
    AX = mybir.AxisListType

    nc = bacc.Bacc("TRN2", target_bir_lowering=False, debug=False)

    # ---- DRAM parameters -------------------------------------------------
    F8 = mybir.dt.float8e4
    K2 = KT // 2              # 256-deep fp8 DoubleRow k-tiles
    xt_d = nc.dram_tensor("xt", [KT, 128, XTP], F32, kind="ExternalInput")
    if mode == "fp8":
        wa_d = nc.dram_tensor("wa8", [K2, 128, 2, H], F8, kind="ExternalInput")
        wb_d = nc.dram_tensor("wb8", [K2, 128, 2, H], F8, kind="ExternalInput")
    else:
        wa_d = nc.dram_tensor("wa", [KT, 128, H], CD, kind="ExternalInput")
        wb_d = nc.dram_tensor("wb", [KT, 128, H], CD, kind="ExternalInput")
    if mode == "fp8":
        w2p_d = nc.dram_tensor("w2p8", [128, 2, 16], F8, kind="ExternalInput")
    else:
        w2p_d = nc.dram_tensor("w2p_t", [128, CT], CD, kind="ExternalInput")
    b1p_d = nc.dram_tensor("b1p_t", [128, CT], F32, kind="ExternalInput")
    mask_d = nc.dram_tensor("addmask", [BC, S], F32, kind="ExternalInput")
    paoh_d = nc.dram_tensor("pa_onehot", [BC, S], F32, kind="ExternalInput")
    e12_d = nc.dram_tensor("e12t", [CT, 128, BC], CD, kind="ExternalInput")
    ws_d = nc.dram_tensor("ws", [CT, 128, H], CD, kind="ExternalInput")
    b1s_d = nc.dram_tensor("b1s_t", [128, CT], F32, kind="ExternalInput")
    w2s_d = nc.dram_tensor("w2s", [CT, 128, A], CD, kind="ExternalInput")
    b2s_d = nc.dram_tensor("b2s_row", [1, A], CD, kind="ExternalInput")
    soh_d = nc.dram_tensor("sym_onehot", [BC, A], F32, kind="ExternalInput")
    clst_d = nc.dram_tensor("clst", [KT, 128, BC], CD, kind="ExternalInput")
    wc1_d = nc.dram_tensor("wc1", [KT, 128, E], CD, kind="ExternalInput")
    bc1_d = nc.dram_tensor("bc1_t", [128, KT], F32, kind="ExternalInput")
    wc2_d = nc.dram_tensor("wc2_t", [128, KT], CD, kind="ExternalInput")
    bc2_d = nc.dram_tensor("bc2_col", [BC, 1], F32, kind="ExternalInput")
    out_d = nc.dram_tensor("out", [BC, 5], F32, kind="ExternalOutput")

    VCT = E // 128  # chan tiles of the critic hidden dim (4)

    with tile.TileContext(nc) as tc:
        with (
            tc.tile_pool(name="weights", bufs=1) as wpool,
            tc.tile_pool(name="hbuf", bufs=1) as hpool,
            tc.tile_pool(name="small", bufs=1) as spool,
            tc.tile_pool(name="strips", bufs=2) as stpool,
            tc.tile_pool(name="psmain", bufs=2, space="PSUM") as psmain,
            tc.tile_pool(name="pssc", bufs=2, space="PSUM") as pssc,
            tc.tile_pool(name="ps3", bufs=2, space="PSUM") as ps3,
        ):
            # ---- symbol head + critic inputs first (their matmuls fill the
            # PE while the big state/weight DMAs stream in) ----------------
            ws_sb = [wpool.tile([128, H], CD, name=f"ws{k}") for k in range(CT)]
            w2s_sb = [wpool.tile([128, A], CD, name=f"w2s{k}") for k in range(CT)]
            e12_sb = [wpool.tile([128, BC], CD, name=f"e12{k}") for k in range(CT)]
            for k in range(CT):
                nc.sync.dma_start(e12_sb[k][:], e12_d[k, :, :])
                nc.sync.dma_start(ws_sb[k][:], ws_d[k, :, :])
                nc.sync.dma_start(w2s_sb[k][:], w2s_d[k, :, :])
            b1s_sb = wpool.tile([128, CT], F32, name="b1s")
            b2s_sb = wpool.tile([1, A], CD, name="b2s")
            soh_sb = wpool.tile([BC, A], F32, name="soh")
            nc.sync.dma_start(b1s_sb[:], b1s_d[:, :])
            nc.sync.dma_start(b2s_sb[:], b2s_d[:, :])
            nc.sync.dma_start(soh_sb[:], soh_d[:, :])
            clst_sb = [wpool.tile([128, BC], CD, name=f"cls{k}") for k in range(KT)]
            wc1_sb = [wpool.tile([128, E], CD, name=f"wc1{k}") for k in range(KT)]
            for k in range(KT):
                nc.sync.dma_start(clst_sb[k][:], clst_d[k, :, :])
                nc.sync.dma_start(wc1_sb[k][:], wc1_d[k, :, :])
            bc1_sb = wpool.tile([128, KT], F32, name="bc1")
            wc2_sb = wpool.tile([128, KT], CD, name="wc2")
            bc2_sb = wpool.tile([BC, 1], F32, name="bc2")
            nc.sync.dma_start(bc1_sb[:], bc1_d[:, :])
            nc.sync.dma_start(wc2_sb[:], wc2_d[:, :])
            nc.sync.dma_start(bc2_sb[:], bc2_d[:, :])
            ones_sb = wpool.tile([1, BC], CD, name="ones")
            nc.vector.memset(ones_sb[:], 1.0)

            outbuf = spool.tile([BC, 5], F32, name="outbuf")
            nc.vector.memset(outbuf[:], 0.0)

            # ---- symbol head ---------------------------------------------
            sh_sb = [spool.tile([128, BC], CD, name=f"sh{ct}") for ct in range(CT)]
            for ct in range(CT):
                p3 = ps3.tile([128, BC], F32, name="p3", tag="p3")
                for k in range(CT):
                    nc.tensor.matmul(
                        p3[:],
                        ws_sb[k][:, ct * 128 : (ct + 1) * 128],
                        e12_sb[k][:],
                        start=(k == 0),
                        stop=(k == CT - 1),
                    )
                nc.scalar.activation(
                    sh_sb[ct][:], p3[:], AF.Relu, bias=b1s_sb[:, ct : ct + 1]
                )
            psl = ps3.tile([BC, A], F32, name="psl", tag="p3")
            for ct in range(CT):
                nc.tensor.matmul(
                    psl[:], sh_sb[ct][:], w2s_sb[ct][:], start=(ct == 0), stop=False
                )
            nc.tensor.matmul(
                psl[:], ones_sb[:], b2s_sb[:], start=False, stop=True
            )
            smy = spool.tile([BC, A], F32, name="smy")
            nc.vector.tensor_copy(smy[:], psl[:])
            mny = spool.tile([BC, 1], F32, name="mny")
            nc.vector.tensor_reduce(mny[:], smy[:], axis=AX.X, op=OP.max, negate=True)
            pey = spool.tile([BC, A], F32, name="pey")
            zsy = spool.tile([BC, 1], F32, name="zsy")
            nc.scalar.activation(
                pey[:], smy[:], AF.Exp, bias=mny[:, 0:1], accum_out=zsy[:]
            )
            p2y = spool.tile([BC, A], F32, name="p2y")
            s2y = spool.tile([BC, 1], F32, name="s2y")
            nc.vector.tensor_mul(p2y[:], pey[:], smy[:])
            nc.vector.tensor_reduce(s2y[:], p2y[:], axis=AX.X, op=OP.add)
            lzy = spool.tile([BC, 1], F32, name="lzy")
            nc.scalar.activation(lzy[:], zsy[:], AF.Ln)
            lsey = spool.tile([BC, 1], F32, name="lsey")
            nc.vector.tensor_sub(lsey[:], lzy[:], mny[:])
            tmpy = spool.tile([BC, A], F32, name="tmpy")
            say = spool.tile([BC, 1], F32, name="say")
            nc.vector.tensor_mul(tmpy[:], smy[:], soh_sb[:])
            nc.vector.tensor_reduce(say[:], tmpy[:], axis=AX.X, op=OP.add)
            rzy = spool.tile([BC, 1], F32, name="rzy")
            nc.vector.reciprocal(rzy[:], zsy[:])
            s2zy = spool.tile([BC, 1], F32, name="s2zy")
            nc.vector.tensor_mul(s2zy[:], s2y[:], rzy[:])
            nc.vector.tensor_sub(outbuf[:, 1:2], say[:], lsey[:])   # logp_sym
            nc.vector.tensor_sub(outbuf[:, 4:5], lsey[:], s2zy[:])  # ent_sym

            # ---- critic ---------------------------------------------------
            hc_sb = [spool.tile([128, BC], CD, name=f"hc{ct}") for ct in range(VCT)]
            for ct in range(VCT):
                pc = ps3.tile([128, BC], F32, name="pc", tag="p3")
                for k in range(KT):
                    nc.tensor.matmul(
                        pc[:],
                        wc1_sb[k][:, ct * 128 : (ct + 1) * 128],
                        clst_sb[k][:],
                        start=(k == 0),
                        stop=(k == KT - 1),
                    )
                nc.scalar.activation(
                    hc_sb[ct][:], pc[:], AF.Relu, bias=bc1_sb[:, ct : ct + 1]
                )
            pv = ps3.tile([BC, 1], F32, name="pv", tag="p3")
            for ct in range(VCT):
                nc.tensor.matmul(
                    pv[:], hc_sb[ct][:], wc2_sb[:, ct : ct + 1],
                    start=(ct == 0), stop=(ct == VCT - 1),
                )
            nc.vector.tensor_add(outbuf[:, 2:3], pv[:], bc2_sb[:])  # val

            # ---- main-path inputs ----------------------------------------
            if mode == "fp8":
                wa_sb = [wpool.tile([128, 2, H], F8, name=f"wa{k}") for k in range(K2)]
                wb_sb = [wpool.tile([128, 2, H], F8, name=f"wb{k}") for k in range(K2)]
                for k in range(K2):
                    nc.sync.dma_start(wa_sb[k][:], wa_d[k, :, :, :])
                    nc.sync.dma_start(wb_sb[k][:], wb_d[k, :, :, :])
            else:
                wa_sb = [wpool.tile([128, H], CD, name=f"wa{k}") for k in range(KT)]
                wb_sb = [wpool.tile([128, H], CD, name=f"wb{k}") for k in range(KT)]
                for k in range(KT):
                    nc.sync.dma_start(wa_sb[k][:], wa_d[k, :, :])
                    nc.sync.dma_start(wb_sb[k][:], wb_d[k, :, :])
            if mode == "fp8":
                w2p_sb = wpool.tile([128, 2, 16], F8, name="w2p")
                nc.sync.dma_start(w2p_sb[:], w2p_d[:, :, :])
            else:
                w2p_sb = wpool.tile([128, CT], CD, name="w2p")
                nc.sync.dma_start(w2p_sb[:], w2p_d[:, :])
            b1p_sb = wpool.tile([128, CT], F32, name="b1p")
            nc.sync.dma_start(b1p_sb[:], b1p_d[:, :])
            mask_sb = wpool.tile([BC, S], F32, name="mask")
            paoh_sb = wpool.tile([BC, S], F32, name="paoh")
            nc.sync.dma_start(mask_sb[:], mask_d[:, :])
            nc.sync.dma_start(paoh_sb[:], paoh_d[:, :])

            # persistent bf16 X^T strips loaded by casting SWDGE DMA, one
            # independent tile per (k, quarter) window (2049 columns: the
            # extra boundary column serves the +1-shifted V operand) so each
            # quarter's matmuls depend only on its own four window DMAs.
            CW = R // NQ  # 2048 columns per window
            xbf = {}
            if mode == "fp8":
                from concourse.tile_rust import add_dep_helper

                XW = CW + 16  # pad the plane stride to a 16-byte multiple
                prev_dma = {}
                for q in range(NQ):
                    for k2 in range(K2):
                        t = wpool.tile([128, 2, XW], F8, name=f"x8_{k2}_{q}")
                        for j in range(2):
                            dma = nc.gpsimd.dma_start(
                                t[:, j, 0 : CW + 1],
                                xt_d[2 * k2 + j, :, q * CW : q * CW + CW + 1],
                            )
                            # serialize each (k2, j) lane across quarters so
                            # quarter 0's windows finish first instead of all
                            # windows round-robining to a simultaneous finish
                            if (k2, j) in prev_dma:
                                add_dep_helper(
                                    dma.ins, prev_dma[(k2, j)].ins, True,
                                    "x window quarter ordering",
                                )
                            prev_dma[(k2, j)] = dma
                        xbf[(k2, q)] = t
            else:
                for q in range(NQ):
                    for k in range(KT):
                        t = wpool.tile([128, CW + 1], CD, name=f"xbf{k}_{q}")
                        nc.gpsimd.dma_start(
                            t[:], xt_d[k, :, q * CW : q * CW + CW + 1]
                        )
                        xbf[(k, q)] = t

            scores_sb = wpool.tile([BC, S], F32, name="scores")

            # ---- main pair-MLP: quarters of 4 row slices ------------------
            for q in range(NQ):
                ps_q = [
                    psmain.tile([128, RS], F32, name=f"ps{j}", tag=f"ps{j}")
                    for j in range(QS)
                ]
                hs = {}
                for ct in range(CT):
                    if mode == "fp8":
                        for w in range(2 * K2):
                            ab, k2 = divmod(w, K2)
                            wsb = (wa_sb if ab == 0 else wb_sb)[k2]
                            for j in range(QS):
                                nc.tensor.matmul(
                                    ps_q[j][:],
                                    wsb[:, :, ct * 128 : (ct + 1) * 128],
                                    xbf[(k2, q)][:, :, j * RS + ab : j * RS + ab + RS],
                                    start=(w == 0),
                                    stop=(w == 2 * K2 - 1),
                                    perf_mode=mybir.MatmulPerfMode.DoubleRow,
                                )
                    else:
                        for w in range(2 * KT):
                            ab, k = divmod(w, KT)
                            wsb = (wa_sb if ab == 0 else wb_sb)[k]
                            for j in range(QS):
                                nc.tensor.matmul(
                                    ps_q[j][:],
                                    wsb[:, ct * 128 : (ct + 1) * 128],
                                    xbf[(k, q)][:, j * RS + ab : j * RS + ab + RS],
                                    start=(w == 0),
                                    stop=(w == 2 * KT - 1),
                                )
                    for j in range(QS):
                        if mode == "fp8":
                            m, jj = divmod(ct, 2)
                            key = (m, j)
                            if key not in hs:
                                hs[key] = hpool.tile(
                                    [128, 2, RS], F8, name=f"h8_{m}_{j}",
                                    tag=f"h8_{m}_{j}",
                                )
                            plane = hs[key][:, jj, :]
                            # split bias+relu ~2:1 DVE:ACT (ACT's fp8 path is
                            # ~2.4x slower per op) so both hide under the PE
                            if (ct * QS + j) % 3 == 2:
                                nc.scalar.activation(
                                    plane, ps_q[j][:], AF.Relu,
                                    bias=b1p_sb[:, ct : ct + 1],
                                )
                            else:
                                nc.vector.tensor_scalar(
                                    plane, ps_q[j][:],
                                    b1p_sb[:, ct : ct + 1], 0.0,
                                    OP.add, OP.max,
                                )
                        else:
                            h = hpool.tile([128, RS], CD, name=f"h{ct}_{j}",
                                           tag=f"h{ct}_{j}")
                            nc.scalar.activation(
                                h[:], ps_q[j][:], AF.Relu,
                                bias=b1p_sb[:, ct : ct + 1],
                            )
                            hs[(ct, j)] = h
                for j in range(QS):
                    rs = QS * q + j
                    psd = pssc.tile([1, RS], F32, name="psd", tag="psd")
                    if mode == "fp8":
                        for m in range(CT // 2):
                            nc.tensor.matmul(
                                psd[:],
                                w2p_sb[:, :, m : m + 1],
                                hs[(m, j)][:, :, :],
                                start=(m == 0),
                                stop=(m == CT // 2 - 1),
                                perf_mode=mybir.MatmulPerfMode.DoubleRow,
                            )
                    else:
                        for ct in range(CT):
                            nc.tensor.matmul(
                                psd[:],
                                w2p_sb[:, ct : ct + 1],
                                hs[(ct, j)][:],
                                start=(ct == 0),
                                stop=(ct == CT - 1),
                            )
                    sstrip = stpool.tile([1, RS], F32, name="sstrip", tag="sstrip")
                    nc.scalar.activation(
                        sstrip[:], psd[:], AF.Copy, bias=0.0,
                        scale=(1.0 / 8192.0 if mode == "fp8" else 1.0),
                    )
                    b, half = rs // 2, rs % 2
                    nc.sync.dma_start(
                        scores_sb[b : b + 1, half * RS : (half + 1) * RS], sstrip[:]
                    )

            # ---- masked log-softmax + entropy over positions -------------
            sm = spool.tile([BC, S], F32, name="sm")
            nc.vector.tensor_add(sm[:], scores_sb[:], mask_sb[:])
            mneg = spool.tile([BC, 1], F32, name="mneg")
            nc.vector.tensor_reduce(mneg[:], sm[:], axis=AX.X, op=OP.max, negate=True)
            pexp = spool.tile([BC, S], F32, name="pexp")
            zsum = spool.tile([BC, 1], F32, name="zsum")
            nc.scalar.activation(
                pexp[:], sm[:], AF.Exp, bias=mneg[:, 0:1], accum_out=zsum[:]
            )
            ps2 = spool.tile([BC, S], F32, name="ps2")
            s2 = spool.tile([BC, 1], F32, name="s2")
            nc.vector.tensor_mul(ps2[:], pexp[:], sm[:])
            nc.vector.tensor_reduce(s2[:], ps2[:], axis=AX.X, op=OP.add)
            logz = spool.tile([BC, 1], F32, name="logz")
            nc.scalar.activation(logz[:], zsum[:], AF.Ln)
            lse = spool.tile([BC, 1], F32, name="lse")
            nc.vector.tensor_sub(lse[:], logz[:], mneg[:])  # logz + max
            tmp = spool.tile([BC, S], F32, name="tmp")
            spa = spool.tile([BC, 1], F32, name="spa")
            nc.vector.tensor_mul(tmp[:], sm[:], paoh_sb[:])
            nc.vector.tensor_reduce(spa[:], tmp[:], axis=AX.X, op=OP.add)
            rz = spool.tile([BC, 1], F32, name="rz")
            nc.vector.reciprocal(rz[:], zsum[:])
            s2z = spool.tile([BC, 1], F32, name="s2z")
            nc.vector.tensor_mul(s2z[:], s2[:], rz[:])
            nc.vector.tensor_sub(outbuf[:, 0:1], spa[:], lse[:])   # logp_pos
            nc.vector.tensor_sub(outbuf[:, 3:4], lse[:], s2z[:])   # ent_pos

            nc.sync.dma_start(out_d[:, :], outbuf[:])

    nc.compile()
    return nc


def _to_cd(arr):
    import ml_dtypes

    return np.ascontiguousarray(arr).astype(ml_dtypes.bfloat16)


FP8_WSCALE = 32.0   # power-of-two prescale keeping fp8 W1p values mid-range
FP8_W2SCALE = 256.0  # prescale for w2p in fp8; scores divided by 32*256 on chip


def _to_f8(arr):
    import ml_dtypes

    return np.ascontiguousarray(arr).astype(ml_dtypes.float8_e4m3)


def kernel(**inputs):
    global LAST_EXEC_NS
    from concourse.bass_utils import run_bass_kernel_spmd

    mode = MODE
    f32 = np.float32
    states = np.asarray(inputs["states"], f32)
    cls_token = np.asarray(inputs["cls_token"], f32)
    W1p = np.asarray(inputs["W1p"], f32)
    b1p = np.asarray(inputs["b1p"], f32)
    w2p = np.asarray(inputs["w2p"], f32)
    W1s = np.asarray(inputs["W1s"], f32)
    b1s = np.asarray(inputs["b1s"], f32)
    W2s = np.asarray(inputs["W2s"], f32)
    b2s = np.asarray(inputs["b2s"], f32)
    Wc1 = np.asarray(inputs["Wc1"], f32)
    bc1 = np.asarray(inputs["bc1"], f32)
    wc2 = np.asarray(inputs["wc2"], f32)
    bc2 = np.asarray(inputs["bc2"], f32)
    lengths = np.asarray(inputs["lengths"])
    position_action = np.asarray(inputs["position_action"])
    symbol_action = np.asarray(inputs["symbol_action"])

    shared = {}
    if mode == "fp8":
        # DoubleRow layout: [k2, p, j, m] = W[256*k2 + 128*j + p, m] * S
        wa4 = W1p[:E].reshape(KT // 2, 2, 128, H).transpose(0, 2, 1, 3)
        wb4 = W1p[E:].reshape(KT // 2, 2, 128, H).transpose(0, 2, 1, 3)
        shared["wa8"] = _to_f8(wa4 * FP8_WSCALE)
        shared["wb8"] = _to_f8(wb4 * FP8_WSCALE)
        w2pm = np.zeros((128, 2, 16), np.float32)  # plane stride padded to 16B
        w2pm[:, :, : CT // 2] = w2p.reshape(CT // 2, 2, 128).transpose(2, 1, 0)
        shared["w2p8"] = _to_f8(w2pm * FP8_W2SCALE)
        shared["b1p_t"] = np.ascontiguousarray(
            b1p.reshape(CT, 128).T * FP8_WSCALE, dtype=f32
        )
    else:
        shared["wa"] = _to_cd(W1p[:E].reshape(KT, 128, H))
        shared["wb"] = _to_cd(W1p[E:].reshape(KT, 128, H))
        shared["w2p_t"] = _to_cd(w2p.reshape(CT, 128).T)
        shared["b1p_t"] = np.ascontiguousarray(b1p.reshape(CT, 128).T, dtype=f32)
    shared.update({
        "ws": _to_cd(W1s.reshape(CT, 128, H)),
        "b1s_t": np.ascontiguousarray(b1s.reshape(CT, 128).T, dtype=f32),
        "w2s": _to_cd(W2s.reshape(CT, 128, A)),
        "b2s_row": _to_cd(b2s.reshape(1, A)),
        "wc1": _to_cd(Wc1.reshape(KT, 128, E)),
        "bc1_t": np.ascontiguousarray(bc1.reshape(KT, 128).T, dtype=f32),
        "wc2_t": _to_cd(wc2.reshape(KT, 128).T),
        "bc2_col": np.full((BC, 1), bc2[0], dtype=f32),
    })

    in_maps = []
    bidx = np.arange(BC)
    tpos = np.arange(S)
    for c in range(NCORES):
        sl = slice(c * BC, (c + 1) * BC)
        st = states[sl]                       # (BC, S, E)
        xt = np.zeros((E, XTP), f32)
        xt[:, :R] = st.reshape(R, E).T
        ln = lengths[sl].astype(np.int64)
        pa = position_action[sl].astype(np.int64)
        sa = symbol_action[sl].astype(np.int64)
        addmask = np.where(tpos[None, :] < (ln - 1)[:, None], 0.0, -1e30)
        pa_onehot = np.zeros((BC, S), f32)
        pa_onehot[bidx, pa] = 1.0
        sym_onehot = np.zeros((BC, A), f32)
        sym_onehot[bidx, sa] = 1.0
        e12 = np.concatenate([st[bidx, pa], st[bidx, pa + 1]], axis=1)  # (BC, 2E)
        m = dict(shared)
        m["xt"] = np.ascontiguousarray(xt.reshape(KT, 128, XTP))
        m["addmask"] = np.ascontiguousarray(addmask, dtype=f32)
        m["pa_onehot"] = pa_onehot
        m["sym_onehot"] = sym_onehot
        m["e12t"] = _to_cd(e12.T.reshape(CT, 128, BC))
        m["clst"] = _to_cd(cls_token[sl].T.reshape(KT, 128, BC))
        in_maps.append(m)

    if mode not in _CACHED:
        _CACHED[mode] = _build(mode)
    nc = _CACHED[mode]

    res = run_bass_kernel_spmd(
        nc, in_maps, core_ids=list(range(NCORES)), trace=TRACE
    )
    LAST_EXEC_NS = res.exec_time_ns

    outs = [np.asarray(res.results[c]["out"]) for c in range(NCORES)]
    full = np.concatenate(outs, axis=0)        # (64, 5)
    return np.ascontiguousarray(full.T, dtype=f32)  # (5, 64)


# revision 23
# speedup vs baseline: 1.1412x; 1.1075x over previous
"""Trainium2 Bass kernel for the ActorCritic ragged-sequence problem.

Strategy
--------
Data-parallel over batch B=64 across 8 NeuronCores (8 batch rows per core,
weights replicated, no collectives; per-core (8,5) outputs are concatenated on
the host).

Per core the dominant work is the position-actor pair-MLP:
    h[b,t] = relu(x_t @ W1a + x_{t+1} @ W1b + b1p);  scores[b,t] = w2p . h[b,t]
computed as weight-stationary matmuls over the flattened 8192 rows:
  - states are passed host-transposed (feature-major); a casting SWDGE DMA
    loads them straight into persistent bf16 X^T strips, so the moving
    operand slices are just free-dim windows and the +1 shift of the
    "second" element of each pair is a one-element slice offset — the PE
    accumulates u_t + v_{t+1} in PSUM for free.
  - the row space is processed in quarters of 4x512 rows so each 128x128
    stationary weight tile is loaded once per 4 matmuls (LDWEIGHTS reuse).
  - ACT applies bias+relu per chan-tile; the w2p dot runs as M=1 matmuls at
    quarter end (LDWEIGHTS is 1 column there, ~free).
Masked log-softmax + entropy run on an (8, 1024) batch-major score tile; the
additive length mask is folded into the PSUM->SBUF strip copy.
Index-derived tensors (masks, one-hots, gathered pair embeddings e1/e2) are
computed on the host from the actual inputs at call time — pure
indexing/layout, no FLOPs moved off-device.  The symbol head and critic are
emitted first so their matmuls fill the PE while the big DMAs stream in.
"""

import os
import numpy as np

B, S, E, A = 64, 1024, 512, 128
NCORES = 8
BC = B // NCORES          # batch rows per core
H = 2 * E                 # pair-MLP hidden dim
R = BC * S                # flattened rows per core
RS = 512                  # row-slice (matmul moving free dim)
NRS = R // RS             # 16 row slices
NQ = 8                    # row-slice groups ("quarters")
QS = NRS // NQ            # row slices per group
KT = E // 128             # 4 k-tiles over the E features
CT = H // 128             # 8 chan tiles of the hidden dim
XTP = R + 8               # padded free dim of the transposed states

MODE = os.environ.get("K_MODE", "bf16")
TRACE = os.environ.get("K_TRACE", "1") == "1"

LAST_EXEC_NS = None
_CACHED = {}

_LDWOPT = os.environ.get("K_LDWOPT", "0") == "1"
_PATCHED = False


def _patch_walrus_flags():
    """Re-enable walrus LDWEIGHTS dedup (repeated stationary operands) for
    this process's compiles."""
    global _PATCHED
    if _PATCHED or not _LDWOPT:
        return
    import concourse.bass_utils as _bu

    _orig = _bu.run_command

    def _rc(argv, **kw):
        argv = [
            "--enable-ldw-opt=true" if a == "--enable-ldw-opt=false" else a
            for a in argv
        ]
        return _orig(argv, **kw)

    _bu.run_command = _rc
    _PATCHED = True


def _build(mode):
    import concourse.tile as tile
    from concourse import bacc, mybir

    _patch_walrus_flags()

    F32 = mybir.dt.float32
    BF16 = mybir.dt.bfloat16
    CD = BF16
    AF = mybir.ActivationFunctionType
    OP = mybir.AluOpType
    AX = mybir.AxisListType

    nc = bacc.Bacc("TRN2", target_bir_lowering=False, debug=False)

    # ---- DRAM parameters -------------------------------------------------
    F8 = mybir.dt.float8e4
    K2 = KT // 2              # 256-deep fp8 DoubleRow k-tiles
    xt_d = nc.dram_tensor("xt", [KT, 128, XTP], F32, kind="ExternalInput")
    if mode == "fp8":
        wa_d = nc.dram_tensor("wa8", [K2, 128, 2, H], F8, kind="ExternalInput")
        wb_d = nc.dram_tensor("wb8", [K2, 128, 2, H], F8, kind="ExternalInput")
    else:
        wa_d = nc.dram_tensor("wa", [KT, 128, H], CD, kind="ExternalInput")
        wb_d = nc.dram_tensor("wb", [KT, 128, H], CD, kind="ExternalInput")
    if mode == "fp8":
        w2p_d = nc.dram_tensor("w2p8", [128, 2, 16], F8, kind="ExternalInput")
    else:
        w2p_d = nc.dram_tensor("w2p_t", [128, CT], CD, kind="ExternalInput")
    b1p_d = nc.dram_tensor("b1p_t", [128, CT], F32, kind="ExternalInput")
    mask_d = nc.dram_tensor("addmask", [BC, S], F32, kind="ExternalInput")
    paoh_d = nc.dram_tensor("pa_onehot", [BC, S], F32, kind="ExternalInput")
    e12_d = nc.dram_tensor("e12t", [CT, 128, BC], CD, kind="ExternalInput")
    ws_d = nc.dram_tensor("ws", [CT, 128, H], CD, kind="ExternalInput")
    b1s_d = nc.dram_tensor("b1s_t", [128, CT], F32, kind="ExternalInput")
    w2s_d = nc.dram_tensor("w2s", [CT, 128, A], CD, kind="ExternalInput")
    b2s_d = nc.dram_tensor("b2s_row", [1, A], CD, kind="ExternalInput")
    soh_d = nc.dram_tensor("sym_onehot", [BC, A], F32, kind="ExternalInput")
    clst_d = nc.dram_tensor("clst", [KT, 128, BC], CD, kind="ExternalInput")
    wc1_d = nc.dram_tensor("wc1", [KT, 128, E], CD, kind="ExternalInput")
    bc1_d = nc.dram_tensor("bc1_t", [128, KT], F32, kind="ExternalInput")
    wc2_d = nc.dram_tensor("wc2_t", [128, KT], CD, kind="ExternalInput")
    bc2_d = nc.dram_tensor("bc2_col", [BC, 1], F32, kind="ExternalInput")
    out_d = nc.dram_tensor("out", [BC, 5], F32, kind="ExternalOutput")

    VCT = E // 128  # chan tiles of the critic hidden dim (4)

    with tile.TileContext(nc) as tc:
        with (
            tc.tile_pool(name="weights", bufs=1) as wpool,
            tc.tile_pool(name="hbuf", bufs=1) as hpool,
            tc.tile_pool(name="small", bufs=1) as spool,
            tc.tile_pool(name="strips", bufs=2) as stpool,
            tc.tile_pool(name="psmain", bufs=2, space="PSUM") as psmain,
            tc.tile_pool(name="pssc", bufs=2, space="PSUM") as pssc,
            tc.tile_pool(name="ps3", bufs=2, space="PSUM") as ps3,
        ):
            # ---- symbol head + critic inputs first (their matmuls fill the
            # PE while the big state/weight DMAs stream in) ----------------
            ws_sb = [wpool.tile([128, H], CD, name=f"ws{k}") for k in range(CT)]
            w2s_sb = [wpool.tile([128, A], CD, name=f"w2s{k}") for k in range(CT)]
            e12_sb = [wpool.tile([128, BC], CD, name=f"e12{k}") for k in range(CT)]
            for k in range(CT):
                nc.sync.dma_start(e12_sb[k][:], e12_d[k, :, :])
                nc.sync.dma_start(ws_sb[k][:], ws_d[k, :, :])
                nc.sync.dma_start(w2s_sb[k][:], w2s_d[k, :, :])
            b1s_sb = wpool.tile([128, CT], F32, name="b1s")
            b2s_sb = wpool.tile([1, A], CD, name="b2s")
            soh_sb = wpool.tile([BC, A], F32, name="soh")
            nc.sync.dma_start(b1s_sb[:], b1s_d[:, :])
            nc.sync.dma_start(b2s_sb[:], b2s_d[:, :])
            nc.sync.dma_start(soh_sb[:], soh_d[:, :])
            clst_sb = [wpool.tile([128, BC], CD, name=f"cls{k}") for k in range(KT)]
            wc1_sb = [wpool.tile([128, E], CD, name=f"wc1{k}") for k in range(KT)]
            for k in range(KT):
                nc.sync.dma_start(clst_sb[k][:], clst_d[k, :, :])
                nc.sync.dma_start(wc1_sb[k][:], wc1_d[k, :, :])
            bc1_sb = wpool.tile([128, KT], F32, name="bc1")
            wc2_sb = wpool.tile([128, KT], CD, name="wc2")
            bc2_sb = wpool.tile([BC, 1], F32, name="bc2")
            nc.sync.dma_start(bc1_sb[:], bc1_d[:, :])
            nc.sync.dma_start(wc2_sb[:], wc2_d[:, :])
            nc.sync.dma_start(bc2_sb[:], bc2_d[:, :])
            ones_sb = wpool.tile([1, BC], CD, name="ones")
            nc.vector.memset(ones_sb[:], 1.0)

            outbuf = spool.tile([BC, 5], F32, name="outbuf")
            nc.vector.memset(outbuf[:], 0.0)

            # ---- symbol head ---------------------------------------------
            sh_sb = [spool.tile([128, BC], CD, name=f"sh{ct}") for ct in range(CT)]
            for ct in range(CT):
                p3 = ps3.tile([128, BC], F32, name="p3", tag="p3")
                for k in range(CT):
                    nc.tensor.matmul(
                        p3[:],
                        ws_sb[k][:, ct * 128 : (ct + 1) * 128],
                        e12_sb[k][:],
                        start=(k == 0),
                        stop=(k == CT - 1),
                    )
                nc.scalar.activation(
                    sh_sb[ct][:], p3[:], AF.Relu, bias=b1s_sb[:, ct : ct + 1]
                )
            psl = ps3.tile([BC, A], F32, name="psl", tag="p3")
            for ct in range(CT):
                nc.tensor.matmul(
                    psl[:], sh_sb[ct][:], w2s_sb[ct][:], start=(ct == 0), stop=False
                )
            nc.tensor.matmul(
                psl[:], ones_sb[:], b2s_sb[:], start=False, stop=True
            )
            smy = spool.tile([BC, A], F32, name="smy")
            nc.vector.tensor_copy(smy[:], psl[:])
            mny = spool.tile([BC, 1], F32, name="mny")
            nc.vector.tensor_reduce(mny[:], smy[:], axis=AX.X, op=OP.max, negate=True)
            pey = spool.tile([BC, A], F32, name="pey")
            zsy = spool.tile([BC, 1], F32, name="zsy")
            nc.scalar.activation(
                pey[:], smy[:], AF.Exp, bias=mny[:, 0:1], accum_out=zsy[:]
            )
            p2y = spool.tile([BC, A], F32, name="p2y")
            s2y = spool.tile([BC, 1], F32, name="s2y")
            nc.vector.tensor_mul(p2y[:], pey[:], smy[:])
            nc.vector.tensor_reduce(s2y[:], p2y[:], axis=AX.X, op=OP.add)
            lzy = spool.tile([BC, 1], F32, name="lzy")
            nc.scalar.activation(lzy[:], zsy[:], AF.Ln)
            lsey = spool.tile([BC, 1], F32, name="lsey")
            nc.vector.tensor_sub(lsey[:], lzy[:], mny[:])
            tmpy = spool.tile([BC, A], F32, name="tmpy")
            say = spool.tile([BC, 1], F32, name="say")
            nc.vector.tensor_mul(tmpy[:], smy[:], soh_sb[:])
            nc.vector.tensor_reduce(say[:], tmpy[:], axis=AX.X, op=OP.add)
            rzy = spool.tile([BC, 1], F32, name="rzy")
            nc.vector.reciprocal(rzy[:], zsy[:])
            s2zy = spool.tile([BC, 1], F32, name="s2zy")
            nc.vector.tensor_mul(s2zy[:], s2y[:], rzy[:])
            nc.vector.tensor_sub(outbuf[:, 1:2], say[:], lsey[:])   # logp_sym
            nc.vector.tensor_sub(outbuf[:, 4:5], lsey[:], s2zy[:])  # ent_sym

            # ---- critic ---------------------------------------------------
            hc_sb = [spool.tile([128, BC], CD, name=f"hc{ct}") for ct in range(VCT)]
            for ct in range(VCT):
                pc = ps3.tile([128, BC], F32, name="pc", tag="p3")
                for k in range(KT):
                    nc.tensor.matmul(
                        pc[:],
                        wc1_sb[k][:, ct * 128 : (ct + 1) * 128],
                        clst_sb[k][:],
                        start=(k == 0),
                        stop=(k == KT - 1),
                    )
                nc.scalar.activation(
                    hc_sb[ct][:], pc[:], AF.Relu, bias=bc1_sb[:, ct : ct + 1]
                )
            pv = ps3.tile([BC, 1], F32, name="pv", tag="p3")
            for ct in range(VCT):
                nc.tensor.matmul(
                    pv[:], hc_sb[ct][:], wc2_sb[:, ct : ct + 1],
                    start=(ct == 0), stop=(ct == VCT - 1),
                )
            nc.vector.tensor_add(outbuf[:, 2:3], pv[:], bc2_sb[:])  # val

            # ---- main-path inputs ----------------------------------------
            if mode == "fp8":
                wa_sb = [wpool.tile([128, 2, H], F8, name=f"wa{k}") for k in range(K2)]
                wb_sb = [wpool.tile([128, 2, H], F8, name=f"wb{k}") for k in range(K2)]
                last_wdma = None
                for k in range(K2):
                    nc.sync.dma_start(wa_sb[k][:], wa_d[k, :, :, :])
                    last_wdma = nc.sync.dma_start(wb_sb[k][:], wb_d[k, :, :, :])
            else:
                wa_sb = [wpool.tile([128, H], CD, name=f"wa{k}") for k in range(KT)]
                wb_sb = [wpool.tile([128, H], CD, name=f"wb{k}") for k in range(KT)]
                for k in range(KT):
                    nc.sync.dma_start(wa_sb[k][:], wa_d[k, :, :])
                    nc.sync.dma_start(wb_sb[k][:], wb_d[k, :, :])
            if mode == "fp8":
                w2p_sb = wpool.tile([128, 2, 16], F8, name="w2p")
                nc.sync.dma_start(w2p_sb[:], w2p_d[:, :, :])
            else:
                w2p_sb = wpool.tile([128, CT], CD, name="w2p")
                nc.sync.dma_start(w2p_sb[:], w2p_d[:, :])
            b1p_sb = wpool.tile([128, CT], F32, name="b1p")
            nc.sync.dma_start(b1p_sb[:], b1p_d[:, :])
            mask_sb = wpool.tile([BC, S], F32, name="mask")
            paoh_sb = wpool.tile([BC, S], F32, name="paoh")
            nc.sync.dma_start(mask_sb[:], mask_d[:, :])
            nc.sync.dma_start(paoh_sb[:], paoh_d[:, :])

            # persistent bf16 X^T strips loaded by casting SWDGE DMA, one
            # independent tile per (k, quarter) window (2049 columns: the
            # extra boundary column serves the +1-shifted V operand) so each
            # quarter's matmuls depend only on its own four window DMAs.
            CW = R // NQ  # 2048 columns per window
            xbf = {}
            if mode == "fp8":
                from concourse.tile_rust import add_dep_helper

                XW = CW + 16  # pad the plane stride to a 16-byte multiple
                prev_dma = {}
                for q in range(NQ):
                    for k2 in range(K2):
                        t = wpool.tile([128, 2, XW], F8, name=f"x8_{k2}_{q}")
                        for j in range(2):
                            dma = nc.gpsimd.dma_start(
                                t[:, j, 0 : CW + 1],
                                xt_d[2 * k2 + j, :, q * CW : q * CW + CW + 1],
                            )
                            # serialize each (k2, j) lane across window groups
                            # so group q's windows land first instead of all
                            # windows round-robining to a simultaneous finish;
                            # gate the whole stream behind the (small) weight
                            # loads so they aren't starved by the x stream
                            if (k2, j) in prev_dma:
                                add_dep_helper(
                                    dma.ins, prev_dma[(k2, j)].ins, True,
                                    "x window group ordering",
                                )
                            elif last_wdma is not None:
                                add_dep_helper(
                                    dma.ins, last_wdma.ins, True,
                                    "x stream starts after weight loads",
                                )
                            prev_dma[(k2, j)] = dma
                        xbf[(k2, q)] = t
            else:
                for q in range(NQ):
                    for k in range(KT):
                        t = wpool.tile([128, CW + 1], CD, name=f"xbf{k}_{q}")
                        nc.gpsimd.dma_start(
                            t[:], xt_d[k, :, q * CW : q * CW + CW + 1]
                        )
                        xbf[(k, q)] = t

            scores_sb = wpool.tile([BC, S], F32, name="scores")

            # ---- main pair-MLP: quarters of 4 row slices ------------------
            for q in range(NQ):
                ps_q = [
                    psmain.tile([128, RS], F32, name=f"ps{j}", tag=f"ps{j}")
                    for j in range(QS)
                ]
                hs = {}
                for ct in range(CT):
                    if mode == "fp8":
                        for w in range(2 * K2):
                            ab, k2 = divmod(w, K2)
                            wsb = (wa_sb if ab == 0 else wb_sb)[k2]
                            for j in range(QS):
                                nc.tensor.matmul(
                                    ps_q[j][:],
                                    wsb[:, :, ct * 128 : (ct + 1) * 128],
                                    xbf[(k2, q)][:, :, j * RS + ab : j * RS + ab + RS],
                                    start=(w == 0),
                                    stop=(w == 2 * K2 - 1),
                                    perf_mode=mybir.MatmulPerfMode.DoubleRow,
                                )
                    else:
                        for w in range(2 * KT):
                            ab, k = divmod(w, KT)
                            wsb = (wa_sb if ab == 0 else wb_sb)[k]
                            for j in range(QS):
                                nc.tensor.matmul(
                                    ps_q[j][:],
                                    wsb[:, ct * 128 : (ct + 1) * 128],
                                    xbf[(k, q)][:, j * RS + ab : j * RS + ab + RS],
                                    start=(w == 0),
                                    stop=(w == 2 * KT - 1),
                                )
                    for j in range(QS):
                        if mode == "fp8":
                            m, jj = divmod(ct, 2)
                            key = (m, j)
                            if key not in hs:
                                hs[key] = hpool.tile(
                                    [128, 2, RS], F8, name=f"h8_{m}_{j}",
                                    tag=f"h8_{m}_{j}",
                                )
                            plane = hs[key][:, jj, :]
                            # split bias+relu ~2:1 DVE:ACT (ACT's fp8 path is
                            # ~2.4x slower per op) so both hide under the PE
                            if (ct * QS + j) % 3 == 2:
                                nc.scalar.activation(
                                    plane, ps_q[j][:], AF.Relu,
                                    bias=b1p_sb[:, ct : ct + 1],
                                )
                            else:
                                nc.vector.tensor_scalar(
                                    plane, ps_q[j][:],
                                    b1p_sb[:, ct : ct + 1], 0.0,
                                    OP.add, OP.max,
                                )
                        else:
                            h = hpool.tile([128, RS], CD, name=f"h{ct}_{j}",
                                           tag=f"h{ct}_{j}")
                            nc.scalar.activation(
                                h[:], ps_q[j][:], AF.Relu,
                                bias=b1p_sb[:, ct : ct + 1],
                            )
                            hs[(ct, j)] = h
                for j in range(QS):
                    rs = QS * q + j
                    psd = pssc.tile([1, RS], F32, name="psd", tag="psd")
                    if mode == "fp8":
                        for m in range(CT // 2):
                            nc.tensor.matmul(
                                psd[:],
                                w2p_sb[:, :, m : m + 1],
                                hs[(m, j)][:, :, :],
                                start=(m == 0),
                                stop=(m == CT // 2 - 1),
                                perf_mode=mybir.MatmulPerfMode.DoubleRow,
                            )
                    else:
                        for ct in range(CT):
                            nc.tensor.matmul(
                                psd[:],
                                w2p_sb[:, ct : ct + 1],
                                hs[(ct, j)][:],
                                start=(ct == 0),
                                stop=(ct == CT - 1),
                            )
                    sstrip = stpool.tile([1, RS], F32, name="sstrip", tag="sstrip")
                    nc.scalar.activation(
                        sstrip[:], psd[:], AF.Copy, bias=0.0,
                        scale=(1.0 / 8192.0 if mode == "fp8" else 1.0),
                    )
                    b, half = rs // 2, rs % 2
                    nc.sync.dma_start(
                        scores_sb[b : b + 1, half * RS : (half + 1) * RS], sstrip[:]
                    )

            # ---- masked log-softmax + entropy over positions -------------
            sm = spool.tile([BC, S], F32, name="sm")
            nc.vector.tensor_add(sm[:], scores_sb[:], mask_sb[:])
            mneg = spool.tile([BC, 1], F32, name="mneg")
            nc.vector.tensor_reduce(mneg[:], sm[:], axis=AX.X, op=OP.max, negate=True)
            pexp = spool.tile([BC, S], F32, name="pexp")
            zsum = spool.tile([BC, 1], F32, name="zsum")
            nc.scalar.activation(
                pexp[:], sm[:], AF.Exp, bias=mneg[:, 0:1], accum_out=zsum[:]
            )
            ps2 = spool.tile([BC, S], F32, name="ps2")
            s2 = spool.tile([BC, 1], F32, name="s2")
            nc.vector.tensor_mul(ps2[:], pexp[:], sm[:])
            nc.vector.tensor_reduce(s2[:], ps2[:], axis=AX.X, op=OP.add)
            logz = spool.tile([BC, 1], F32, name="logz")
            nc.scalar.activation(logz[:], zsum[:], AF.Ln)
            lse = spool.tile([BC, 1], F32, name="lse")
            nc.vector.tensor_sub(lse[:], logz[:], mneg[:])  # logz + max
            tmp = spool.tile([BC, S], F32, name="tmp")
            spa = spool.tile([BC, 1], F32, name="spa")
            nc.vector.tensor_mul(tmp[:], sm[:], paoh_sb[:])
            nc.vector.tensor_reduce(spa[:], tmp[:], axis=AX.X, op=OP.add)
            rz = spool.tile([BC, 1], F32, name="rz")
            nc.vector.reciprocal(rz[:], zsum[:])
            s2z = spool.tile([BC, 1], F32, name="s2z")
            nc.vector.tensor_mul(s2z[:], s2[:], rz[:])
            nc.vector.tensor_sub(outbuf[:, 0:1], spa[:], lse[:])   # logp_pos
            nc.vector.tensor_sub(outbuf[:, 3:4], lse[:], s2z[:])   # ent_pos

            nc.sync.dma_start(out_d[:, :], outbuf[:])

    nc.compile()
    return nc


def _to_cd(arr):
    import ml_dtypes

    return np.ascontiguousarray(arr).astype(ml_dtypes.bfloat16)


FP8_WSCALE = 32.0   # power-of-two prescale keeping fp8 W1p values mid-range
FP8_W2SCALE = 256.0  # prescale for w2p in fp8; scores divided by 32*256 on chip


def _to_f8(arr):
    import ml_dtypes

    return np.ascontiguousarray(arr).astype(ml_dtypes.float8_e4m3)


def kernel(**inputs):
    global LAST_EXEC_NS
    from concourse.bass_utils import run_bass_kernel_spmd

    mode = MODE
    f32 = np.float32
    states = np.asarray(inputs["states"], f32)
    cls_token = np.asarray(inputs["cls_token"], f32)
    W1p = np.asarray(inputs["W1p"], f32)
    b1p = np.asarray(inputs["b1p"], f32)
    w2p = np.asarray(inputs["w2p"], f32)
    W1s = np.asarray(inputs["W1s"], f32)
    b1s = np.asarray(inputs["b1s"], f32)
    W2s = np.asarray(inputs["W2s"], f32)
    b2s = np.asarray(inputs["b2s"], f32)
    Wc1 = np.asarray(inputs["Wc1"], f32)
    bc1 = np.asarray(inputs["bc1"], f32)
    wc2 = np.asarray(inputs["wc2"], f32)
    bc2 = np.asarray(inputs["bc2"], f32)
    lengths = np.asarray(inputs["lengths"])
    position_action = np.asarray(inputs["position_action"])
    symbol_action = np.asarray(inputs["symbol_action"])

    shared = {}
    if mode == "fp8":
        # DoubleRow layout: [k2, p, j, m] = W[256*k2 + 128*j + p, m] * S
        wa4 = W1p[:E].reshape(KT // 2, 2, 128, H).transpose(0, 2, 1, 3)
        wb4 = W1p[E:].reshape(KT // 2, 2, 128, H).transpose(0, 2, 1, 3)
        shared["wa8"] = _to_f8(wa4 * FP8_WSCALE)
        shared["wb8"] = _to_f8(wb4 * FP8_WSCALE)
        w2pm = np.zeros((128, 2, 16), np.float32)  # plane stride padded to 16B
        w2pm[:, :, : CT // 2] = w2p.reshape(CT // 2, 2, 128).transpose(2, 1, 0)
        shared["w2p8"] = _to_f8(w2pm * FP8_W2SCALE)
        shared["b1p_t"] = np.ascontiguousarray(
            b1p.reshape(CT, 128).T * FP8_WSCALE, dtype=f32
        )
    else:
        shared["wa"] = _to_cd(W1p[:E].reshape(KT, 128, H))
        shared["wb"] = _to_cd(W1p[E:].reshape(KT, 128, H))
        shared["w2p_t"] = _to_cd(w2p.reshape(CT, 128).T)
        shared["b1p_t"] = np.ascontiguousarray(b1p.reshape(CT, 128).T, dtype=f32)
    shared.update({
        "ws": _to_cd(W1s.reshape(CT, 128, H)),
        "b1s_t": np.ascontiguousarray(b1s.reshape(CT, 128).T, dtype=f32),
        "w2s": _to_cd(W2s.reshape(CT, 128, A)),
        "b2s_row": _to_cd(b2s.reshape(1, A)),
        "wc1": _to_cd(Wc1.reshape(KT, 128, E)),
        "bc1_t": np.ascontiguousarray(bc1.reshape(KT, 128).T, dtype=f32),
        "wc2_t": _to_cd(wc2.reshape(KT, 128).T),
        "bc2_col": np.full((BC, 1), bc2[0], dtype=f32),
    })

    in_maps = []
    bidx = np.arange(BC)
    tpos = np.arange(S)
    for c in range(NCORES):
        sl = slice(c * BC, (c + 1) * BC)
        st = states[sl]                       # (BC, S, E)
        xt = np.zeros((E, XTP), f32)
        xt[:, :R] = st.reshape(R, E).T
        ln = lengths[sl].astype(np.int64)
        pa = position_action[sl].astype(np.int64)
        sa = symbol_action[sl].astype(np.int64)
        addmask = np.where(tpos[None, :] < (ln - 1)[:, None], 0.0, -1e30)
        pa_onehot = np.zeros((BC, S), f32)
        pa_onehot[bidx, pa] = 1.0
        sym_onehot = np.zeros((BC, A), f32)
        sym_onehot[bidx, sa] = 1.0
        e12 = np.concatenate([st[bidx, pa], st[bidx, pa + 1]], axis=1)  # (BC, 2E)
        m = dict(shared)
        m["xt"] = np.ascontiguousarray(xt.reshape(KT, 128, XTP))
        m["addmask"] = np.ascontiguousarray(addmask, dtype=f32)
        m["pa_onehot"] = pa_onehot
        m["sym_onehot"] = sym_onehot
        m["e12t"] = _to_cd(e12.T.reshape(CT, 128, BC))
        m["clst"] = _to_cd(cls_token[sl].T.reshape(KT, 128, BC))
        in_maps.append(m)

    if mode not in _CACHED:
        _CACHED[mode] = _build(mode)
    nc = _CACHED[mode]

    res = run_bass_kernel_spmd(
        nc, in_maps, core_ids=list(range(NCORES)), trace=TRACE
    )
    LAST_EXEC_NS = res.exec_time_ns

    outs = [np.asarray(res.results[c]["out"]) for c in range(NCORES)]
    full = np.concatenate(outs, axis=0)        # (64, 5)
    return np.ascontiguousarray(full.T, dtype=f32)  # (5, 64)
